# revision 33
# baseline (speedup 1.0000x reference)
"""Single-head attention with QKV projections on 8 TRN2 NeuronCores.

Problem: B=4, S=2048, E=A=1024 f32.
  q = query @ Wq + bq ; k = key @ Wk + bk ; v = value @ Wv + bv
  out = softmax(q k^T / sqrt(A)) v

Sharding: data-parallel over (batch, query-half) -> 8 shards. Each core
computes the K projection for its whole batch (duplicated across the core
pair; a pair AllGather is ~85us, far larger than the ~27us of PE time it
would save given scores^T needs kT immediately), but the V projection is
deduplicated: each core projects only its own 1024 rows of V and a pair
AllGather assembles the full V in DRAM while the ~110us scores^T phase
runs, hiding the collective entirely.

Layout strategy (per core):
  - The host pre-transposes activations (and casts operands to bf16) so every
    matmul contracts over the partition axis with zero on-chip transposes:
    xq = query_shard^T [E, 1024], xk = key_b^T, xv = value_b^T.
  - Projections produce qT [A, Sq] and kT-chunks [A, 512] (A on partitions)
    and v [Sk, A] (natural).
  - Scores are computed TRANSPOSED: sT[k, q] = kT_chunk^T @ qT, so that
    E = exp(sT/sqrt(A)) (bf16) is directly the lhsT of the probs @ V matmul -
    no transpose of the probability matrix and no partition-axis softmax
    reductions. The row-max subtraction is skipped (|scores| <= ~6 for this
    input distribution; exp is safe in f32), making the softmax a plain
    exp/sum. Softmax denominators: GpSimd accumulates acc = sum_kt E[kt]
    while scores stream, then 8 tiny f32 matmuls acc[:, qs]^T @ ones give
    per-partition denominators; 1/denom is folded into the PSUM->SBUF copy
    of the output. v-bias is added at the very end (sum_k probs = 1).
  - All matmul operands are bf16 (PSUM accumulation is f32; measured
    rel_l2 vs the f32 reference ~5.4e-3). bf16 also halves input DMA and
    enables the fast weight load path.

Phase order A (q-proj) -> B (v-half proj, AllGather issued) -> Cs (fused
k-proj chunk -> scores^T -> exp, AllGather completes underneath) -> AV. Weight tensors live in separate single-buffer pools whose ungated
DMAs are all issued up front on the Scalar HWDGE queue (keeping them off the
Sync queue avoids head-of-line blocking of the xk/xv streams); activations
stream on Sync/Scalar with >=2KB per-partition rows for DMA packet
efficiency. Long-lived tensors (qT, v, E, acc) are raw SBUF allocations
because pool lifetimes are strictly LIFO. Measured: ~258us HW exec, PE busy ~88% with
median matmul issue gap at the 216ns streaming floor.
"""
import sys

sys.path.insert(0, "/opt/trn_rl_repo")

import ml_dtypes
import numpy as np

BF16 = ml_dtypes.bfloat16

import concourse.bass as bass
import concourse.tile as tile
from concourse import bacc, bass_utils, mybir

B, S, E, A = 4, 2048, 1024, 1024
SQ = 1024          # queries per core
ET, AT = 8, 8      # 128-tiles of E and A
ST, KT, KC = 16, 16, 4  # 128-tiles of Sk; k-chunks of 512
QC, QS, AC = 2, 8, 2    # q 512-chunks, q 128-subtiles, a 512-chunks
SCALE = 1.0 / 32.0      # 1/sqrt(A)

f32 = mybir.dt.float32
f32r = mybir.dt.float32r
bf16 = mybir.dt.bfloat16
ts = bass.ts


def build():
    nc = bacc.Bacc("TRN2", target_bir_lowering=False, debug=False,
                   dynamic_dma_scratch_size=8192)
    Act = mybir.ActivationFunctionType
    Alu = mybir.AluOpType

    xq_d = nc.dram_tensor("xq", [E, SQ], bf16, kind="ExternalInput")
    xk_d = nc.dram_tensor("xk", [E, S], bf16, kind="ExternalInput")
    xv_d = nc.dram_tensor("xv", [E, SQ], bf16, kind="ExternalInput")
    wq_d = nc.dram_tensor("wq", [E, A], bf16, kind="ExternalInput")
    wk_d = nc.dram_tensor("wk", [E, A], bf16, kind="ExternalInput")
    wv_d = nc.dram_tensor("wv", [E, A], bf16, kind="ExternalInput")
    bqt_d = nc.dram_tensor("bqt", [128, AT], f32, kind="ExternalInput")
    bkt_d = nc.dram_tensor("bkt", [128, AT], f32, kind="ExternalInput")
    bvb_d = nc.dram_tensor("bvb", [128, A], f32, kind="ExternalInput")
    ones_d = nc.dram_tensor("ones", [128, 2], f32, kind="ExternalInput")
    out_d = nc.dram_tensor("out", [SQ, A], f32, kind="ExternalOutput")

    # Long-lived activations as raw (non-pool) SBUF tensors (pool lifetimes
    # are strictly LIFO; these span multiple phase scopes).
    qT = nc.alloc_sbuf_tensor("qT_sb", [128, AT, SQ], bf16).ap()
    v_sb = nc.alloc_sbuf_tensor("v_sb", [128, ST, A], bf16).ap()
    acc = nc.alloc_sbuf_tensor("acc_sb", [128, SQ], f32).ap()
    recip = nc.alloc_sbuf_tensor("recip_sb", [128, QS], f32).ap()
    ones_t = nc.alloc_sbuf_tensor("ones_sb", [128, 2], f32).ap()

    # Phase order: A (q-proj) -> Cs (fused k-proj + scores^T + exp) ->
    # B (v-proj) -> AV. Cs is the longest PE stretch and provides the DMA
    # window that hides the Wv/xv prefetches; A's window only has to cover
    # wq+xq (8MB ~ its own compute time).
    with tile.TileContext(nc) as tc:
        with (
            tc.tile_pool(name="pp512", bufs=4, space="PSUM") as pp512,
            tc.tile_pool(name="pps", bufs=2, space="PSUM") as pps,
            tc.tile_pool(name="pdram", bufs=1, space="DRAM") as pdram,
        ):
            ag_in = pdram.tile([SQ, A], bf16)       # this core's v half
            ag_out = pdram.tile([S, A], bf16)       # pair-gathered full v
            pe = tc.alloc_tile_pool(name="pe", bufs=1)
            E_t = pe.tile([128, KT, SQ], bf16)  # exp(scores^T) [k, kt, q]
            pwk = tc.alloc_tile_pool(name="pwk", bufs=1)
            pwv = tc.alloc_tile_pool(name="pwv", bufs=1)
            pW = tc.alloc_tile_pool(name="pW", bufs=1)

            # ---- Phase A: qT[a, q] = (query @ Wq + bq)^T ----
            wq = pW.tile([128, ET, A], bf16, tag="w", name="wq_t")
            for et in range(ET):
                nc.scalar.dma_start(wq[:, et, :], wq_d.ap()[ts(et, 128), :])
            pxq = tc.alloc_tile_pool(name="pxq", bufs=1)
            bqt = pxq.tile([128, AT], f32, tag="bqt")
            nc.gpsimd.dma_start(bqt[:], bqt_d.ap()[:, :])
            xq_t = pxq.tile([128, ET, SQ], bf16)
            for et in range(ET):
                nc.sync.dma_start(xq_t[:, et, :], xq_d.ap()[ts(et, 128), :])
            nc.gpsimd.dma_start(ones_t[:], ones_d.ap()[:, :])
            wv = pwv.tile([128, ET, A], bf16)
            for et in range(ET):
                nc.scalar.dma_start(wv[:, et, :], wv_d.ap()[ts(et, 128), :])
            wk = pwk.tile([128, ET, A], bf16)
            for et in range(ET):
                nc.scalar.dma_start(wk[:, et, :], wk_d.ap()[ts(et, 128), :])

            for at in range(AT):
                for qc in range(QC):
                    ps = pp512.tile([128, 512], f32, tag="ps", name="ps_a")
                    for et in range(ET):
                        nc.tensor.matmul(
                            ps[:], wq[:, et, ts(at, 128)],
                            xq_t[:, et, ts(qc, 512)],
                            start=(et == 0), stop=(et == ET - 1),
                        )
                    nc.vector.tensor_scalar(
                        qT[:, at, ts(qc, 512)], ps[:], bqt[:, at:at + 1],
                        None, Alu.add)

            # ---- Phase B: v-half = value_half @ Wv (this core's 1024 rows);
            #      pair AllGather assembles the full v during phase Cs ----
            pxv = tc.alloc_tile_pool(name="pxv", bufs=2)
            pvst = tc.alloc_tile_pool(name="pvst", bufs=2)
            for sc in range(2):          # 512-wide column chunks of the half
                xv_c = pxv.tile([128, ET, 512], bf16, tag="xv", name="xv_c")
                for et in range(ET):
                    nc.scalar.dma_start(
                        xv_c[:, et, :], xv_d.ap()[ts(et, 128), ts(sc, 512)])
                for sti in range(4):
                    stl = sc * 4 + sti   # local s-tile 0..7
                    for ac in range(AC):
                        ps = pp512.tile([128, 512], f32, tag="ps", name="ps_b")
                        for et in range(ET):
                            nc.tensor.matmul(
                                ps[:], xv_c[:, et, ts(sti, 128)],
                                wv[:, et, ts(ac, 512)],
                                start=(et == 0), stop=(et == ET - 1),
                            )
                        vst = pvst.tile([128, 512], bf16, tag="vst", name="vst")
                        nc.scalar.copy(vst[:], ps[:])
                        nc.sync.dma_start(
                            ag_in[ts(stl, 128), ts(ac, 512)], vst[:])
            nc.gpsimd.collective_compute(
                "AllGather",
                Alu.bypass,
                ins=[ag_in.opt()],
                outs=[ag_out.opt()],
                replica_groups=[[0, 1], [2, 3], [4, 5], [6, 7]],
            )

            # ---- Phase Cs: per 512-k-chunk: kT-proj -> scores^T -> exp ----
            if True:
                pcs = tc.alloc_tile_pool(name="pcs", bufs=1)
                pxk = tc.alloc_tile_pool(name="pxk", bufs=2)
                pkc = tc.alloc_tile_pool(name="pkc", bufs=2)
                bkt = pcs.tile([128, AT], f32, tag="bkt")
                nc.gpsimd.dma_start(bkt[:], bkt_d.ap()[:, :])

                for kc in range(KC):
                    xk_t = pxk.tile([128, ET, 512], bf16, tag="xk", name="xk_t")
                    for et in range(ET):
                        nc.sync.dma_start(
                            xk_t[:, et, :], xk_d.ap()[ts(et, 128), ts(kc, 512)])
                    kc_t = pkc.tile([128, AT, 512], bf16, tag="kc", name="kc_t")
                    for at in range(AT):
                        ps = pp512.tile([128, 512], f32, tag="ps", name="ps_k")
                        for et in range(ET):
                            nc.tensor.matmul(
                                ps[:], wk[:, et, ts(at, 128)], xk_t[:, et, :],
                                start=(et == 0), stop=(et == ET - 1),
                            )
                        nc.vector.tensor_scalar(
                            kc_t[:, at, :], ps[:], bkt[:, at:at + 1],
                            None, Alu.add)
                    for ki in range(4):
                        kt = kc * 4 + ki
                        psc = pps.tile([128, SQ], f32, tag="psc", name="psc")
                        for at in range(AT):
                            for qc in range(QC):
                                nc.tensor.matmul(
                                    psc[:, ts(qc, 512)],
                                    kc_t[:, at, ts(ki, 128)],
                                    qT[:, at, ts(qc, 512)],
                                    start=(at == 0), stop=(at == AT - 1),
                                )
                        nc.scalar.activation(
                            E_t[:, kt, :], psc[:], Act.Exp,
                            bias=0.0, scale=SCALE)
                        # denominator partial-sums ride along on DVE
                        if kt == 1:
                            nc.gpsimd.tensor_tensor(
                                acc[:], E_t[:, 0, :], E_t[:, 1, :], Alu.add)
                        elif kt > 1:
                            nc.gpsimd.tensor_tensor(
                                acc[:], acc[:], E_t[:, kt, :], Alu.add)



            # load the AllGathered v into SBUF (runs during late Cs once the
            # pair AllGather completes)
            for st in range(ST):
                nc.sync.dma_start(v_sb[:, st, :], ag_out[ts(st, 128), :])

            # ---- Phase AV: out = (probs @ v) * recip + bv ----
            if True:
                pcm = tc.alloc_tile_pool(name="pcm", bufs=1)
                pot = tc.alloc_tile_pool(name="pot", bufs=2)
                bvb = pcm.tile([128, A], f32)
                nc.gpsimd.dma_start(bvb[:], bvb_d.ap()[:, :])
                first_group = [True]
                for ac in range(AC):
                    for qs in range(QS):
                        ps = pp512.tile([128, 512], f32, tag="ps", name="ps_av")
                        for kt in range(KT):
                            nc.tensor.matmul(
                                ps[:], E_t[:, kt, ts(qs, 128)],
                                v_sb[:, kt, ts(ac, 512)],
                                start=(kt == 0), stop=(kt == KT - 1),
                            )
                        if first_group[0]:
                            # denominators: emitted here so the first AV
                            # group's matmuls cover the acc-chain tail
                            first_group[0] = False
                            for dq in range(QS):
                                psd = pp512.tile([128, 2], f32, tag="ps",
                                                 name="psd")
                                nc.tensor.matmul(
                                    psd[:], acc[:, ts(dq, 128)], ones_t[:],
                                    start=True, stop=True)
                                nc.vector.reciprocal(
                                    recip[:, dq:dq + 1], psd[:, 0:1])
                        ot = pot.tile([128, 512], f32, tag="ot", name="ot")
                        nc.vector.tensor_scalar(
                            ot[:], ps[:], recip[:, qs:qs + 1], None, Alu.mult)
                        nc.vector.tensor_tensor(
                            ot[:], ot[:], bvb[:, ts(ac, 512)], Alu.add)
                        nc.sync.dma_start(
                            out_d.ap()[ts(qs, 128), ts(ac, 512)], ot[:])

            for p in (pot, pcm, pkc, pxk, pcs, pvst, pxv, pxq,
                      pW, pwv, pwk, pe):
                p.release()

    nc.compile()
    return nc


_nc_cache = None


def _get_nc():
    global _nc_cache
    if _nc_cache is None:
        _nc_cache = build()
    return _nc_cache


def kernel(query, key, value, Wq, bq, Wk, bk, Wv, bv):
    query = np.asarray(query, dtype=np.float32)
    key = np.asarray(key, dtype=np.float32)
    value = np.asarray(value, dtype=np.float32)
    Wq = np.ascontiguousarray(np.asarray(Wq, dtype=np.float32))
    Wk = np.ascontiguousarray(np.asarray(Wk, dtype=np.float32))
    Wv = np.ascontiguousarray(np.asarray(Wv, dtype=np.float32))
    bq = np.asarray(bq, dtype=np.float32)
    bk = np.asarray(bk, dtype=np.float32)
    bv = np.asarray(bv, dtype=np.float32)

    nc = _get_nc()

    Wq16 = Wq.astype(BF16)
    Wk16 = Wk.astype(BF16)
    Wv16 = Wv.astype(BF16)
    bqt = np.ascontiguousarray(bq.reshape(AT, 128).T)
    bkt = np.ascontiguousarray(bk.reshape(AT, 128).T)
    bvb = np.ascontiguousarray(np.broadcast_to(bv, (128, A)))
    ones = np.ones((128, 2), np.float32)

    kTs = [np.ascontiguousarray(key[b].T.astype(BF16)) for b in range(B)]

    in_maps = []
    for c in range(8):
        b, h = c // 2, c % 2
        in_maps.append({
            "xq": np.ascontiguousarray(
                query[b, h * SQ:(h + 1) * SQ, :].T.astype(BF16)),
            "xk": kTs[b],
            "xv": np.ascontiguousarray(
                value[b, h * SQ:(h + 1) * SQ, :].T.astype(BF16)),
            "wq": Wq16, "wk": Wk16, "wv": Wv16,
            "bqt": bqt, "bkt": bkt, "bvb": bvb, "ones": ones,
        })

    global _last_in_maps
    _last_in_maps = in_maps
    res = bass_utils.run_bass_kernel_spmd(nc, in_maps, core_ids=list(range(8)))

    out = np.empty((B, S, A), np.float32)
    for c in range(8):
        b, h = c // 2, c % 2
        out[b, h * SQ:(h + 1) * SQ, :] = res.results[c]["out"]
    return out



# revision 35
# speedup vs baseline: 1.0520x; 1.0520x over previous
"""Single-head attention with QKV projections on 8 TRN2 NeuronCores.

Problem: B=4, S=2048, E=A=1024 f32.
  q = query @ Wq + bq ; k = key @ Wk + bk ; v = value @ Wv + bv
  out = softmax(q k^T / sqrt(A)) v

Sharding: data-parallel over (batch, query-half) -> 8 shards. Each core
computes the K projection for its whole batch (duplicated across the core
pair; a pair AllGather is ~85us, far larger than the ~27us of PE time it
would save given scores^T needs kT immediately), but the V projection is
deduplicated: each core projects only its own 1024 rows of V and a pair
AllGather assembles the full V in DRAM while the ~110us scores^T phase
runs, hiding the collective entirely.

Layout strategy (per core):
  - The host pre-transposes activations (and casts operands to bf16) so every
    matmul contracts over the partition axis with zero on-chip transposes:
    xq = query_shard^T [E, 1024], xk = key_b^T, xv = value_b^T.
  - Projections produce qT [A, Sq] and kT-chunks [A, 512] (A on partitions)
    and v [Sk, A] (natural).
  - Scores are computed TRANSPOSED: sT[k, q] = kT_chunk^T @ qT, so that
    E = exp(sT/sqrt(A)) (bf16) is directly the lhsT of the probs @ V matmul -
    no transpose of the probability matrix and no partition-axis softmax
    reductions. The row-max subtraction is skipped (|scores| <= ~6 for this
    input distribution; exp is safe in f32), making the softmax a plain
    exp/sum. Softmax denominators: GpSimd accumulates acc = sum_kt E[kt]
    while scores stream, then 8 tiny f32 matmuls acc[:, qs]^T @ ones give
    per-partition denominators; 1/denom is folded into the PSUM->SBUF copy
    of the output. v-bias is added at the very end (sum_k probs = 1).
  - All matmul operands are bf16 (PSUM accumulation is f32; measured
    rel_l2 vs the f32 reference ~5.4e-3). bf16 also halves input DMA and
    enables the fast weight load path.

Phase order A (q-proj) -> B (v-half proj, AllGather issued) -> Cs (fused
k-proj chunk -> scores^T -> exp, AllGather completes underneath) -> AV. Weight tensors live in separate single-buffer pools whose ungated
DMAs are all issued up front on the Scalar HWDGE queue (keeping them off the
Sync queue avoids head-of-line blocking of the xk/xv streams); activations
stream on Sync/Scalar with >=2KB per-partition rows for DMA packet
efficiency. Long-lived tensors (qT, v, E, acc) are raw SBUF allocations
because pool lifetimes are strictly LIFO. Measured: ~258us HW exec, PE busy ~88% with
median matmul issue gap at the 216ns streaming floor.
"""
import sys

sys.path.insert(0, "/opt/trn_rl_repo")

import ml_dtypes
import numpy as np

BF16 = ml_dtypes.bfloat16

import concourse.bass as bass
import concourse.tile as tile
from concourse import bacc, bass_utils, mybir

B, S, E, A = 4, 2048, 1024, 1024
SQ = 1024          # queries per core
ET, AT = 8, 8      # 128-tiles of E and A
ST, KT, KC = 16, 16, 4  # 128-tiles of Sk; k-chunks of 512
QC, QS, AC = 2, 8, 2    # q 512-chunks, q 128-subtiles, a 512-chunks
SCALE = 1.0 / 32.0      # 1/sqrt(A)

f32 = mybir.dt.float32
f32r = mybir.dt.float32r
bf16 = mybir.dt.bfloat16
ts = bass.ts


def build():
    nc = bacc.Bacc("TRN2", target_bir_lowering=False, debug=False,
                   dynamic_dma_scratch_size=8192)
    Act = mybir.ActivationFunctionType
    Alu = mybir.AluOpType

    xq_d = nc.dram_tensor("xq", [E, SQ], bf16, kind="ExternalInput")
    xk_d = nc.dram_tensor("xk", [E, S], bf16, kind="ExternalInput")
    xv_d = nc.dram_tensor("xv", [E, SQ], bf16, kind="ExternalInput")
    wq_d = nc.dram_tensor("wq", [E, A], bf16, kind="ExternalInput")
    wk_d = nc.dram_tensor("wk", [E, A], bf16, kind="ExternalInput")
    wv_d = nc.dram_tensor("wv", [E, A], bf16, kind="ExternalInput")
    bqt_d = nc.dram_tensor("bqt", [128, AT], f32, kind="ExternalInput")
    bkt_d = nc.dram_tensor("bkt", [128, AT], f32, kind="ExternalInput")
    bvb_d = nc.dram_tensor("bvb", [128, A], f32, kind="ExternalInput")
    ones_d = nc.dram_tensor("ones", [128, 2], f32, kind="ExternalInput")
    out_d = nc.dram_tensor("out", [SQ, A], f32, kind="ExternalOutput")

    # Long-lived activations as raw (non-pool) SBUF tensors (pool lifetimes
    # are strictly LIFO; these span multiple phase scopes).
    qT = nc.alloc_sbuf_tensor("qT_sb", [128, AT, SQ], bf16).ap()
    v_sb = nc.alloc_sbuf_tensor("v_sb", [128, ST, A], bf16).ap()
    acc = nc.alloc_sbuf_tensor("acc_sb", [128, SQ], f32).ap()
    recip = nc.alloc_sbuf_tensor("recip_sb", [128, QS], f32).ap()
    ones_t = nc.alloc_sbuf_tensor("ones_sb", [128, 2], f32).ap()

    # Phase order: A (q-proj) -> Cs (fused k-proj + scores^T + exp) ->
    # B (v-proj) -> AV. Cs is the longest PE stretch and provides the DMA
    # window that hides the Wv/xv prefetches; A's window only has to cover
    # wq+xq (8MB ~ its own compute time).
    with tile.TileContext(nc) as tc:
        with (
            tc.tile_pool(name="pp512", bufs=4, space="PSUM") as pp512,
            tc.tile_pool(name="pps", bufs=2, space="PSUM") as pps,
            tc.tile_pool(name="pdram", bufs=1, space="DRAM") as pdram,
        ):
            ag_in = pdram.tile([SQ, A], bf16)       # this core's v half
            ag_out = pdram.tile([S, A], bf16)       # pair-gathered full v
            pe = tc.alloc_tile_pool(name="pe", bufs=1)
            E_t = pe.tile([128, KT, SQ], bf16)  # exp(scores^T) [k, kt, q]
            pwk = tc.alloc_tile_pool(name="pwk", bufs=1)
            pwv = tc.alloc_tile_pool(name="pwv", bufs=1)
            pW = tc.alloc_tile_pool(name="pW", bufs=1)

            # ---- Phase A: qT[a, q] = (query @ Wq + bq)^T ----
            wq = pW.tile([128, ET, A], bf16, tag="w", name="wq_t")
            for et in range(ET):
                nc.scalar.dma_start(wq[:, et, :], wq_d.ap()[ts(et, 128), :])
            pxq = tc.alloc_tile_pool(name="pxq", bufs=1)
            bqt = pxq.tile([128, AT], f32, tag="bqt")
            nc.gpsimd.dma_start(bqt[:], bqt_d.ap()[:, :])
            xq_t = pxq.tile([128, ET, SQ], bf16)
            for et in range(ET):
                nc.sync.dma_start(xq_t[:, et, :], xq_d.ap()[ts(et, 128), :])
            nc.gpsimd.dma_start(ones_t[:], ones_d.ap()[:, :])
            wv = pwv.tile([128, ET, A], bf16)
            for et in range(ET):
                nc.scalar.dma_start(wv[:, et, :], wv_d.ap()[ts(et, 128), :])
            wk = pwk.tile([128, ET, A], bf16)
            for et in range(ET):
                nc.scalar.dma_start(wk[:, et, :], wk_d.ap()[ts(et, 128), :])

            # Phases A and B are data-independent; interleaving their matmul
            # groups lets the merged ~40us compute window absorb the startup
            # DMA pacing of both input streams.
            pxv = tc.alloc_tile_pool(name="pxv", bufs=2)
            pvst = tc.alloc_tile_pool(name="pvst", bufs=2)
            def a_group(at, qc):
                ps = pp512.tile([128, 512], f32, tag="ps", name="ps_a")
                for et in range(ET):
                    nc.tensor.matmul(
                        ps[:], wq[:, et, ts(at, 128)],
                        xq_t[:, et, ts(qc, 512)],
                        start=(et == 0), stop=(et == ET - 1),
                    )
                nc.vector.tensor_scalar(
                    qT[:, at, ts(qc, 512)], ps[:], bqt[:, at:at + 1],
                    None, Alu.add)

            def b_group(sc, sti, ac, xv_c):
                stl = sc * 4 + sti
                ps = pp512.tile([128, 512], f32, tag="ps", name="ps_b")
                for et in range(ET):
                    nc.tensor.matmul(
                        ps[:], xv_c[:, et, ts(sti, 128)],
                        wv[:, et, ts(ac, 512)],
                        start=(et == 0), stop=(et == ET - 1),
                    )
                vst = pvst.tile([128, 512], bf16, tag="vst", name="vst")
                nc.scalar.copy(vst[:], ps[:])
                nc.sync.dma_start(ag_in[ts(stl, 128), ts(ac, 512)], vst[:])

            xv_tiles = {}
            for sc in range(2):
                xv_c = pxv.tile([128, ET, 512], bf16, tag="xv",
                                name=f"xv_c{sc}", bufs=2)
                for et in range(ET):
                    nc.scalar.dma_start(
                        xv_c[:, et, :], xv_d.ap()[ts(et, 128), ts(sc, 512)])
                xv_tiles[sc] = xv_c
            a_work = [(at, qc) for at in range(AT) for qc in range(QC)]
            b_work = [(sc, sti, ac) for sc in range(2) for sti in range(4)
                      for ac in range(AC)]
            for i in range(16):
                a_group(*a_work[i])
                sc, sti, ac = b_work[i]
                b_group(sc, sti, ac, xv_tiles[sc])

            nc.gpsimd.collective_compute(
                "AllGather",
                Alu.bypass,
                ins=[ag_in.opt()],
                outs=[ag_out.opt()],
                replica_groups=[[0, 1], [2, 3], [4, 5], [6, 7]],
            )

            # ---- Phase Cs: per 512-k-chunk: kT-proj -> scores^T -> exp ----
            if True:
                pcs = tc.alloc_tile_pool(name="pcs", bufs=1)
                pxk = tc.alloc_tile_pool(name="pxk", bufs=2)
                pkc = tc.alloc_tile_pool(name="pkc", bufs=2)
                bkt = pcs.tile([128, AT], f32, tag="bkt")
                nc.gpsimd.dma_start(bkt[:], bkt_d.ap()[:, :])

                for kc in range(KC):
                    xk_t = pxk.tile([128, ET, 512], bf16, tag="xk", name="xk_t")
                    for et in range(ET):
                        nc.sync.dma_start(
                            xk_t[:, et, :], xk_d.ap()[ts(et, 128), ts(kc, 512)])
                    kc_t = pkc.tile([128, AT, 512], bf16, tag="kc", name="kc_t")
                    for at in range(AT):
                        ps = pp512.tile([128, 512], f32, tag="ps", name="ps_k")
                        for et in range(ET):
                            nc.tensor.matmul(
                                ps[:], wk[:, et, ts(at, 128)], xk_t[:, et, :],
                                start=(et == 0), stop=(et == ET - 1),
                            )
                        nc.vector.tensor_scalar(
                            kc_t[:, at, :], ps[:], bkt[:, at:at + 1],
                            None, Alu.add)
                    for ki in range(4):
                        kt = kc * 4 + ki
                        psc = pps.tile([128, SQ], f32, tag="psc", name="psc")
                        for at in range(AT):
                            for qc in range(QC):
                                nc.tensor.matmul(
                                    psc[:, ts(qc, 512)],
                                    kc_t[:, at, ts(ki, 128)],
                                    qT[:, at, ts(qc, 512)],
                                    start=(at == 0), stop=(at == AT - 1),
                                )
                        nc.scalar.activation(
                            E_t[:, kt, :], psc[:], Act.Exp,
                            bias=0.0, scale=SCALE)
                        # denominator partial-sums ride along on DVE
                        if kt == 1:
                            nc.gpsimd.tensor_tensor(
                                acc[:], E_t[:, 0, :], E_t[:, 1, :], Alu.add)
                        elif kt > 1:
                            nc.gpsimd.tensor_tensor(
                                acc[:], acc[:], E_t[:, kt, :], Alu.add)



            # load the AllGathered v into SBUF (runs during late Cs once the
            # pair AllGather completes)
            for st in range(ST):
                nc.sync.dma_start(v_sb[:, st, :], ag_out[ts(st, 128), :])

            # ---- Phase AV: out = (probs @ v) * recip + bv ----
            if True:
                pcm = tc.alloc_tile_pool(name="pcm", bufs=1)
                pot = tc.alloc_tile_pool(name="pot", bufs=2)
                bvb = pcm.tile([128, A], f32)
                nc.gpsimd.dma_start(bvb[:], bvb_d.ap()[:, :])
                first_group = [True]
                for ac in range(AC):
                    for qs in range(QS):
                        ps = pp512.tile([128, 512], f32, tag="ps", name="ps_av")
                        for kt in range(KT):
                            nc.tensor.matmul(
                                ps[:], E_t[:, kt, ts(qs, 128)],
                                v_sb[:, kt, ts(ac, 512)],
                                start=(kt == 0), stop=(kt == KT - 1),
                            )
                        if first_group[0]:
                            # denominators: emitted here so the first AV
                            # group's matmuls cover the acc-chain tail
                            first_group[0] = False
                            for dq in range(QS):
                                psd = pp512.tile([128, 2], f32, tag="ps",
                                                 name="psd")
                                nc.tensor.matmul(
                                    psd[:], acc[:, ts(dq, 128)], ones_t[:],
                                    start=True, stop=True)
                                nc.vector.reciprocal(
                                    recip[:, dq:dq + 1], psd[:, 0:1])
                        ot = pot.tile([128, 512], f32, tag="ot", name="ot")
                        nc.vector.tensor_scalar(
                            ot[:], ps[:], recip[:, qs:qs + 1], None, Alu.mult)
                        nc.vector.tensor_tensor(
                            ot[:], ot[:], bvb[:, ts(ac, 512)], Alu.add)
                        nc.sync.dma_start(
                            out_d.ap()[ts(qs, 128), ts(ac, 512)], ot[:])

            for p in (pot, pcm, pkc, pxk, pcs, pvst, pxv, pxq,
                      pW, pwv, pwk, pe):
                p.release()

    nc.compile()
    return nc


_nc_cache = None


def _get_nc():
    global _nc_cache
    if _nc_cache is None:
        _nc_cache = build()
    return _nc_cache


def kernel(query, key, value, Wq, bq, Wk, bk, Wv, bv):
    query = np.asarray(query, dtype=np.float32)
    key = np.asarray(key, dtype=np.float32)
    value = np.asarray(value, dtype=np.float32)
    Wq = np.ascontiguousarray(np.asarray(Wq, dtype=np.float32))
    Wk = np.ascontiguousarray(np.asarray(Wk, dtype=np.float32))
    Wv = np.ascontiguousarray(np.asarray(Wv, dtype=np.float32))
    bq = np.asarray(bq, dtype=np.float32)
    bk = np.asarray(bk, dtype=np.float32)
    bv = np.asarray(bv, dtype=np.float32)

    nc = _get_nc()

    Wq16 = Wq.astype(BF16)
    Wk16 = Wk.astype(BF16)
    Wv16 = Wv.astype(BF16)
    bqt = np.ascontiguousarray(bq.reshape(AT, 128).T)
    bkt = np.ascontiguousarray(bk.reshape(AT, 128).T)
    bvb = np.ascontiguousarray(np.broadcast_to(bv, (128, A)))
    ones = np.ones((128, 2), np.float32)

    kTs = [np.ascontiguousarray(key[b].T.astype(BF16)) for b in range(B)]

    in_maps = []
    for c in range(8):
        b, h = c // 2, c % 2
        in_maps.append({
            "xq": np.ascontiguousarray(
                query[b, h * SQ:(h + 1) * SQ, :].T.astype(BF16)),
            "xk": kTs[b],
            "xv": np.ascontiguousarray(
                value[b, h * SQ:(h + 1) * SQ, :].T.astype(BF16)),
            "wq": Wq16, "wk": Wk16, "wv": Wv16,
            "bqt": bqt, "bkt": bkt, "bvb": bvb, "ones": ones,
        })

    global _last_in_maps
    _last_in_maps = in_maps
    res = bass_utils.run_bass_kernel_spmd(nc, in_maps, core_ids=list(range(8)))

    out = np.empty((B, S, A), np.float32)
    for c in range(8):
        b, h = c // 2, c % 2
        out[b, h * SQ:(h + 1) * SQ, :] = res.results[c]["out"]
    return out

